# revision 1
# baseline (speedup 1.0000x reference)
"""InnerProductDecoder GNN edge-scoring kernel for 8 TRN2 NeuronCores.

Math: out[e] = (sigmoid(w * z[s]@(c@psi)[d]) + sigmoid(w * (c@psi)[s]@z[d])) / 2
Key identity: (c@psi)[s] . z[d] == c[s] . (z@psi.T)[d], so with zt = z@psi.T
both per-edge dots are K=64 dots against the packed table u = [c | zt] (N x 128
f32, 512B rows):
    v_cz[e] = u[s,0:64] . u[d,64:128]
    v_zc[e] = u[s,64:128] . u[d,0:64]

Per core: build the full u table in DRAM (PE matmul zt = z@psi.T), then for its
1/8 slice of edges dma_gather u[src], u[dst] (512B rows), DVE dot, ACT sigmoid.
dma_gather uses int16 indices, so the node table is split in two 25000-row
windows and edges are bucketed into 4 classes by (src-half, dst-half); each
class gathers with window-relative indices. Edge order is restored on host.
"""
import numpy as np

import concourse.bass as bass
import concourse.tile as tile
from concourse import bacc, mybir
from concourse.bass_utils import run_bass_kernel_spmd

N, D, K, E = 50000, 128, 64, 600000
NCORES = 8
HALF = N // 2          # int16 gather window size
EPC = E // NCORES      # edges per core
G = 2048               # edges per gather chunk
F32 = mybir.dt.float32
I16 = mybir.dt.int16


def _pack_idx(arr: np.ndarray) -> np.ndarray:
    """Gather-index layout: idx i -> partition i%16, col i//16; replicated 8x."""
    n = arr.shape[0]
    t = arr.astype(np.int16).reshape(n // 16, 16).T
    return np.tile(t, (8, 1))


def _build_bass(C: list[int]):
    """C[c] = padded per-class edge count (multiple of 128, same on all cores)."""
    TOT = sum(C)
    TOT16, TOTJ = TOT // 16, TOT // 128

    nc = bacc.Bacc("TRN2", target_bir_lowering=False, debug=False,
                   num_devices=NCORES)
    zt_in = nc.dram_tensor("zt", [D, N], F32, kind="ExternalInput")
    u_tab_t = nc.dram_tensor("utab", [N, D], F32, kind="ExternalInput")
    psit_in = nc.dram_tensor("psit", [D, K], F32, kind="ExternalInput")
    w_in = nc.dram_tensor("w", [1, 1], F32, kind="ExternalInput")
    s_in = nc.dram_tensor("sidx", [128, TOT16], I16, kind="ExternalInput")
    d_in = nc.dram_tensor("didx", [128, TOT16], I16, kind="ExternalInput")
    out = nc.dram_tensor("out", [128, TOTJ], F32, kind="ExternalOutput")

    with tile.TileContext(nc) as tc:
        with (
            tc.tile_pool(name="const", bufs=1) as cpool,
            tc.tile_pool(name="blda", bufs=3) as apool,
            tc.tile_pool(name="bldu", bufs=3) as upool,
            tc.tile_pool(name="psum", bufs=3, space="PSUM") as ppool,
            tc.tile_pool(name="gat", bufs=3) as gpool,
            tc.tile_pool(name="mul", bufs=3) as mpool,
            tc.tile_pool(name="red", bufs=3) as rpool,
        ):
            # --- constants ---
            psit_t = cpool.tile([D, K], F32)
            nc.sync.dma_start(psit_t[:], psit_in.ap())
            w_t = cpool.tile([1, 1], F32)
            nc.sync.dma_start(w_t[:], w_in.ap())
            w_b = cpool.tile([128, 1], F32)
            nc.gpsimd.partition_broadcast(w_b[:], w_t[:])
            sidx_t = cpool.tile([128, TOT16], I16)
            nc.sync.dma_start(sidx_t[:], s_in.ap())
            didx_t = cpool.tile([128, TOT16], I16)
            nc.sync.dma_start(didx_t[:], d_in.ap())
            out_sb = cpool.tile([128, TOTJ], F32)

            # --- phase A: write zt half into pre-filled u table ---
            u_tab = u_tab_t.ap()
            GR = 1024  # rows per build group
            r0 = 0
            while r0 < N:
                rows = min(GR, N - r0)
                nfull = rows // 128          # full 128-row sub-tiles
                rem = rows - nfull * 128     # tail rows (< 128)
                zt_blk = apool.tile([128, GR], F32, tag="zt")
                nc.sync.dma_start(zt_blk[:, :rows], zt_in.ap()[:, r0:r0 + rows])
                u_big = upool.tile([128, GR // 128, K], F32, tag="ub")
                ps = ppool.tile([128, 512], F32, tag="ps")
                nsub = nfull + (1 if rem else 0)
                for g in range(nsub):
                    sr = 128 if g < nfull else rem
                    nc.tensor.matmul(
                        out=ps[:sr, g * K:(g + 1) * K],
                        lhsT=zt_blk[:, g * 128:g * 128 + sr],
                        rhs=psit_t[:],
                        start=True, stop=True,
                    )
                # copy zt into SBUF, then write zt half-rows (bytes 256:512)
                if nfull:
                    nc.vector.tensor_copy(
                        u_big[:, :nfull, :],
                        ps[:, 0:nfull * K].rearrange("p (g k) -> p g k", k=K),
                    )
                    nc.sync.dma_start(
                        u_tab[r0:r0 + nfull * 128, K:D].rearrange(
                            "(g p) d -> p g d", p=128),
                        u_big[:, :nfull, :],
                    )
                if rem:
                    nc.vector.tensor_copy(
                        u_big[:rem, nfull, :], ps[:rem, nfull * K:(nfull + 1) * K])
                    nc.sync.dma_start(
                        u_tab[r0 + nfull * 128:r0 + rows, K:D],
                        u_big[:rem, nfull, :],
                    )
                r0 += rows

            # --- phase B: gather + dot + sigmoid ---
            col = 0   # running offset (in edges) into idx/out arrays
            for cls in range(4):
                ws, wd = cls >> 1, cls & 1
                u_s = u_tab[ws * HALF:(ws + 1) * HALF, :]
                u_d = u_tab[wd * HALF:(wd + 1) * HALF, :]
                done = 0
                while done < C[cls]:
                    g = min(G, C[cls] - done)
                    j = g // 128
                    c16, cj = col // 16, col // 128
                    s_t = gpool.tile([128, G // 128, D], F32, tag="st")
                    nc.gpsimd.dma_gather(
                        s_t[:, :j, :], u_s, sidx_t[:, c16:c16 + g // 16],
                        num_idxs=g, num_idxs_reg=g, elem_size=D, single_packet=False)
                    d_t = gpool.tile([128, G // 128, D], F32, tag="dt")
                    nc.gpsimd.dma_gather(
                        d_t[:, :j, :], u_d, didx_t[:, c16:c16 + g // 16],
                        num_idxs=g, num_idxs_reg=g, elem_size=D, single_packet=False)
                    m1 = mpool.tile([128, G // 128, K], F32, tag="m1")
                    nc.vector.tensor_tensor(
                        out=m1[:, :j, :], in0=s_t[:, :j, 0:K], in1=d_t[:, :j, K:D],
                        op=mybir.AluOpType.mult)
                    m2 = mpool.tile([128, G // 128, K], F32, tag="m2")
                    nc.vector.tensor_tensor(
                        out=m2[:, :j, :], in0=s_t[:, :j, K:D], in1=d_t[:, :j, 0:K],
                        op=mybir.AluOpType.mult)
                    r1 = rpool.tile([128, G // 128], F32, tag="r1")
                    nc.vector.tensor_reduce(
                        out=r1[:, :j], in_=m1[:, :j, :], axis=mybir.AxisListType.X,
                        op=mybir.AluOpType.add)
                    r2 = rpool.tile([128, G // 128], F32, tag="r2")
                    nc.vector.tensor_reduce(
                        out=r2[:, :j], in_=m2[:, :j, :], axis=mybir.AxisListType.X,
                        op=mybir.AluOpType.add)
                    sg1 = rpool.tile([128, G // 128], F32, tag="sg1")
                    nc.scalar.activation(
                        sg1[:, :j], r1[:, :j], mybir.ActivationFunctionType.Sigmoid,
                        scale=w_b[:])
                    sg2 = rpool.tile([128, G // 128], F32, tag="sg2")
                    nc.scalar.activation(
                        sg2[:, :j], r2[:, :j], mybir.ActivationFunctionType.Sigmoid,
                        scale=w_b[:])
                    sm = rpool.tile([128, G // 128], F32, tag="sm")
                    nc.vector.tensor_tensor(
                        out=sm[:, :j], in0=sg1[:, :j], in1=sg2[:, :j],
                        op=mybir.AluOpType.add)
                    nc.vector.tensor_scalar(
                        out=out_sb[:, cj:cj + j], in0=sm[:, :j],
                        scalar1=0.5, scalar2=None, op0=mybir.AluOpType.mult)
                    done += g
                    col += g

            nc.sync.dma_start(out.ap(), out_sb[:])
    nc.compile()
    return nc


def prepare(z, c, psi, weights, edge_index):
    z = np.asarray(z, dtype=np.float32)
    c = np.asarray(c, dtype=np.float32)
    psi = np.asarray(psi, dtype=np.float32)
    weights = np.asarray(weights, dtype=np.float32)
    ei = np.asarray(edge_index).astype(np.int64)

    zt = np.ascontiguousarray(z.T)                    # [D, N]
    psit = np.ascontiguousarray(psi.T)                # [D, K]
    w = weights.reshape(1, 1)
    utab_init = np.zeros((N, D), dtype=np.float32)
    utab_init[:, 0:K] = c

    # --- host: bucket each core's edges into 4 (src-half, dst-half) classes ---
    src_all = ei[0].astype(np.int32)
    dst_all = ei[1].astype(np.int32)
    per_core = []
    counts = np.zeros((NCORES, 4), dtype=np.int64)
    for i in range(NCORES):
        s = src_all[i * EPC:(i + 1) * EPC]
        d = dst_all[i * EPC:(i + 1) * EPC]
        cls = ((s >= HALF).astype(np.int32) << 1) | (d >= HALF).astype(np.int32)
        order = np.argsort(cls, kind="stable")
        per_core.append((s, d, cls, order))
        for cc in range(4):
            counts[i, cc] = int((cls == cc).sum())
    C = [int(-(-counts[:, cc].max() // 128) * 128) for cc in range(4)]

    nc = _build_bass(C)

    in_maps = []
    for i in range(NCORES):
        s, d, cls, order = per_core[i]
        s_sorted, d_sorted, cls_sorted = s[order], d[order], cls[order]
        s_seg, d_seg = [], []
        base = 0
        for cc in range(4):
            n = int(counts[i, cc])
            pad = C[cc] - n
            ws, wd = cc >> 1, cc & 1
            s_rel = np.concatenate([s_sorted[base:base + n] - ws * HALF,
                                    np.zeros(pad, dtype=np.int32)])
            d_rel = np.concatenate([d_sorted[base:base + n] - wd * HALF,
                                    np.zeros(pad, dtype=np.int32)])
            s_seg.append(_pack_idx(s_rel))
            d_seg.append(_pack_idx(d_rel))
            base += n
        in_maps.append({
            "zt": zt, "utab": utab_init, "psit": psit, "w": w,
            "sidx": np.ascontiguousarray(np.concatenate(s_seg, axis=1)),
            "didx": np.ascontiguousarray(np.concatenate(d_seg, axis=1)),
        })

    return nc, in_maps, (per_core, counts, C)


def unshard(results, meta):
    per_core, counts, C = meta
    final = np.empty(E, dtype=np.float32)
    offj = np.cumsum([0] + [cc // 128 for cc in C])
    for i in range(NCORES):
        s, d, cls, order = per_core[i]
        dev = results[i]["out"]                # [128, TOTJ]
        base = 0
        for cc in range(4):
            n = int(counts[i, cc])
            blk = dev[:, offj[cc]:offj[cc + 1]]          # [128, C[cc]//128]
            vals = blk.T.ravel()[:n]
            final[i * EPC + order[base:base + n]] = vals
            base += n
    return final


def kernel(z, c, psi, weights, edge_index):
    nc, in_maps, meta = prepare(z, c, psi, weights, edge_index)
    res = run_bass_kernel_spmd(nc, in_maps, core_ids=list(range(NCORES)))
    kernel.last_results = res
    return unshard(res.results, meta)



# revision 6
# speedup vs baseline: 1.4381x; 1.4381x over previous
"""InnerProductDecoder edge-scoring kernel ("Design G v2") for 8 TRN2 cores.

Math: out[e] = (sigmoid(w*zt[s].c[d]) + sigmoid(w*c[s].zt[d]))/2, zt = z@psi^T.

Per core (~75k edges):
- bf16 node tables: u_sb = [c|zt] node-major in SBUF (PE weights); u2a/u2b =
  [zt|c] in DRAM (SWDGE transpose-gather source, split by dst window so
  window-0 gathers overlap the phase-A build of window 1 rows).
- src side: no gather. Edges slotted by (dst_window, src_block) with
  per-bucket round-robin dealing across cores -> static shared slot ranges.
  Per 512-col psum subtile, a few matmuls with lhsT=u_sb[block] and
  rhs=host-built one-hot columns select the src rows.
- dst side: one SWDGE transpose-gather descriptor per edge (feat-major bf16
  columns), round-robined over 4 SWDGE queues.
- dot: DVE bf16 mult; feat reduce via PE ones-block matmul -> psum [2,n];
  ACT sigmoid (scale=w); PE [.5,.5] matmul adds the pair; ACT evac to DRAM.
"""
import numpy as np
import ml_dtypes

import concourse.bass as bass
import concourse.tile as tile
from concourse import bacc, mybir
from concourse.bass_utils import run_bass_kernel_spmd

N, D, K, E = 50000, 128, 64, 600000
NP, NB = 50048, 391          # padded nodes = 128*391
NCORES = 8
WN0 = 32768                  # dst window split (idx = dst & 32767)
NB0 = WN0 // 128             # 256 blocks in window 0
SUB = 512                    # psum subtile columns
GCH = 4096                   # dst-gather chunk (8 subtiles)
GB = 16                      # phase-A blocks per group
F32 = mybir.dt.float32
BF16 = mybir.dt.bfloat16
I16 = mybir.dt.int16
BF = ml_dtypes.bfloat16


def _pack_idx16(arr: np.ndarray) -> np.ndarray:
    n = arr.shape[0]
    t = arr.astype(np.int16).reshape(n // 16, 16).T
    return np.tile(t, (8, 1))


def _layout(counts):
    """counts[w][b] = per-core slots for bucket. Static layout shared by all
    cores: ranges, window spans, subtiles (with block col-ranges), chunks."""
    ranges = {}
    wspan = []
    off = 0
    for w in range(2):
        wstart = off
        for b in range(NB):
            r = counts[w][b]
            if r:
                ranges[(w, b)] = (off, r)
            off += r
        off = -(-off // SUB) * SUB
        wspan.append((wstart, off))
    tot = off
    subtiles = []
    for w in range(2):
        ws, we = wspan[w]
        items = sorted((v[0], v[0] + v[1], b)
                       for (ww, b), v in ranges.items() if ww == w)
        for col0 in range(ws, we, SUB):
            blks = []
            for s, e2, b in items:
                c0, c1 = max(s, col0), min(e2, col0 + SUB)
                if c0 < c1:
                    blks.append((b, c0, c1))
            cov = sum(c1 - c0 for _, c0, c1 in blks)
            if cov < SUB:           # cover pad columns (zero one-hots)
                cols = np.zeros(SUB, dtype=bool)
                for _, c0, c1 in blks:
                    cols[c0 - col0:c1 - col0] = True
                i = 0
                while i < SUB:
                    if cols[i]:
                        i += 1
                        continue
                    j = i
                    while j < SUB and not cols[j]:
                        j += 1
                    blks.append((0, col0 + i, col0 + j))
                    i = j
                blks.sort(key=lambda t: t[1])
            subtiles.append((col0, w, blks))
    chunks = []
    for w in range(2):
        ws, we = wspan[w]
        c = ws
        while c < we:
            n = min(GCH, we - c)
            chunks.append((c, n, w))
            c += n
    return tot, wspan, subtiles, chunks


def _build(tot, wspan, subtiles, chunks):
    nc = bacc.Bacc("TRN2", target_bir_lowering=False, debug=False,
                   num_devices=NCORES, num_swdge_queues=4)
    zT_in = nc.dram_tensor("zT", [128, NP], BF16, kind="ExternalInput")
    psiT_in = nc.dram_tensor("psiT", [128, 64], BF16, kind="ExternalInput")
    uc_in = nc.dram_tensor("uc", [NP, 64], BF16, kind="ExternalInput")
    u2a_t = nc.dram_tensor("u2a", [WN0, 128], BF16, kind="ExternalInput")
    u2b_t = nc.dram_tensor("u2b", [NP - WN0, 128], BF16,
                           kind="ExternalInput")
    oh_in = nc.dram_tensor("oh", [128, tot], BF16, kind="ExternalInput")
    didx_in = nc.dram_tensor("didx", [128, tot // 16], I16,
                             kind="ExternalInput")
    w_in = nc.dram_tensor("w", [1, 1], F32, kind="ExternalInput")
    out_t = nc.dram_tensor("out", [1, tot], F32, kind="ExternalOutput")

    with tile.TileContext(nc) as tc:
        with (
            tc.tile_pool(name="const", bufs=1) as cpool,
            tc.tile_pool(name="zstream", bufs=3) as zpool,
            tc.tile_pool(name="dtp", bufs=3) as dpool,
            tc.tile_pool(name="chk", bufs=2) as kpool,
            tc.tile_pool(name="sub", bufs=3) as spool,
            tc.tile_pool(name="pa", bufs=2, space="PSUM") as papool,
            tc.tile_pool(name="ps", bufs=2, space="PSUM") as pspool,
            tc.tile_pool(name="pr", bufs=2, space="PSUM") as prpool,
            tc.tile_pool(name="pf", bufs=2, space="PSUM") as pfpool,
        ):
            psiT = cpool.tile([128, 64], BF16)
            nc.sync.dma_start(psiT[:], psiT_in.ap())
            w_t = cpool.tile([1, 1], F32)
            nc.sync.dma_start(w_t[:], w_in.ap())
            w_b = cpool.tile([128, 1], F32)
            nc.gpsimd.partition_broadcast(w_b[:], w_t[:])
            bones = cpool.tile([128, 2], BF16)
            nc.gpsimd.memset(bones[:], 0.0)
            nc.gpsimd.memset(bones[0:64, 0:1], 1.0)
            nc.gpsimd.memset(bones[64:128, 1:2], 1.0)
            half = cpool.tile([2, 1], BF16)
            nc.gpsimd.memset(half[:], 0.5)
            didx_t = cpool.tile([128, tot // 16], I16)
            nc.sync.dma_start(didx_t[:], didx_in.ap())
            u_sb = cpool.tile([128, NB, 128], BF16)

            def phase_a_group(g):
                gb = min(GB, NB - g)
                rows = gb * 128
                r0 = g * 128
                nc.sync.dma_start(
                    u_sb[:, g:g + gb, 0:64],
                    uc_in.ap()[r0:r0 + rows, :].rearrange(
                        "(a p) k -> p a k", p=128))
                zsl = zpool.tile([128, GB * 128], BF16, tag="z")
                nc.sync.dma_start(zsl[:, :rows], zT_in.ap()[:, r0:r0 + rows])
                for h in range(0, gb, 8):
                    hb = min(8, gb - h)
                    pa = pspool.tile([128, 512], F32, tag="ps")
                    for k in range(hb):
                        nc.tensor.matmul(
                            out=pa[:, k * 64:(k + 1) * 64],
                            lhsT=zsl[:, (h + k) * 128:(h + k + 1) * 128],
                            rhs=psiT[:],
                            start=True, stop=True)
                    nc.scalar.activation(
                        u_sb[:, g + h:g + h + hb, 64:128],
                        pa[:, :hb * 64].rearrange("p (a k) -> p a k", k=64),
                        mybir.ActivationFunctionType.Copy)
                if r0 < WN0:
                    dst_ap = u2a_t.ap()[r0:r0 + rows, 0:64]
                else:
                    dst_ap = u2b_t.ap()[r0 - WN0:r0 - WN0 + rows, 0:64]
                nc.sync.dma_start(
                    dst_ap.rearrange("(a p) k -> p a k", p=128),
                    u_sb[:, g:g + gb, 64:128])

            gath = {}

            def emit_gather(ci):
                ccol, ncols, w = chunks[ci]
                src_t = u2a_t if w == 0 else u2b_t
                d_t = dpool.tile([128, 1, GCH], BF16, tag="dt")
                nc.gpsimd.dma_gather(
                    d_t[:, :, :ncols], src_t.ap(),
                    didx_t[:, ccol // 16:(ccol + ncols) // 16],
                    num_idxs=ncols, num_idxs_reg=ncols, elem_size=128,
                    transpose=True, single_packet=False,
                    queue_num=ci % 4)
                gath[ci] = d_t

            # phase A for window-0 rows, then first gathers, then the rest
            for g in range(0, NB0, GB):
                phase_a_group(g)
            emit_gather(0)
            emit_gather(1)
            for g in range(NB0, NB, GB):
                phase_a_group(g)

            sub_by_col = {s[0]: s for s in subtiles}
            for ci, (ccol, ncols, w) in enumerate(chunks):
                if ci + 2 < len(chunks):
                    emit_gather(ci + 2)
                d_t = gath.pop(ci)
                oh_t = kpool.tile([128, GCH], BF16, tag="oh")
                nc.sync.dma_start(oh_t[:, :ncols],
                                  oh_in.ap()[:, ccol:ccol + ncols])
                fo = kpool.tile([1, GCH], F32, tag="fo")
                for sc in range(ccol, ccol + ncols, SUB):
                    _, _, blks = sub_by_col[sc]
                    rel = sc - ccol
                    ps = pspool.tile([128, SUB], F32, tag="ps")
                    for (b, c0, c1) in blks:
                        nc.tensor.matmul(
                            out=ps[:, c0 - sc:c1 - sc],
                            lhsT=u_sb[:, b, :],
                            rhs=oh_t[:, c0 - ccol:c1 - ccol],
                            start=True, stop=True)
                    s_bf = spool.tile([128, SUB], BF16, tag="sbf")
                    nc.vector.tensor_copy(s_bf[:], ps[:])
                    m = spool.tile([128, SUB], BF16, tag="m")
                    nc.vector.tensor_tensor(
                        out=m[:], in0=s_bf[:],
                        in1=d_t[:, 0, rel:rel + SUB],
                        op=mybir.AluOpType.mult)
                    pr = prpool.tile([2, SUB], F32, tag="pr")
                    nc.tensor.matmul(
                        out=pr[:], lhsT=bones[:], rhs=m[:],
                        start=True, stop=True)
                    sg = spool.tile([2, SUB], BF16, tag="sg")
                    nc.scalar.activation(
                        sg[:], pr[:], mybir.ActivationFunctionType.Sigmoid,
                        scale=w_b[0:2])
                    pf = pfpool.tile([1, SUB], F32, tag="pf")
                    nc.tensor.matmul(
                        out=pf[:], lhsT=half[:], rhs=sg[:],
                        start=True, stop=True)
                    nc.scalar.activation(
                        fo[:, rel:rel + SUB], pf[:],
                        mybir.ActivationFunctionType.Copy)
                nc.sync.dma_start(out_t.ap()[:, ccol:ccol + ncols],
                                  fo[:, :ncols])
    nc.compile()
    return nc


def prepare(z, c, psi, weights, edge_index):
    z = np.asarray(z, dtype=np.float32)
    c = np.asarray(c, dtype=np.float32)
    psi = np.asarray(psi, dtype=np.float32)
    weights = np.asarray(weights, dtype=np.float32).reshape(1, 1)
    ei = np.asarray(edge_index).astype(np.int64)
    src = ei[0].astype(np.int32)
    dst = ei[1].astype(np.int32)

    wof = (dst >= WN0).astype(np.int32)
    bof = src // 128
    key = wof * NB + bof
    order = np.argsort(key, kind="stable")
    ksort = key[order]
    bounds = np.searchsorted(ksort, np.arange(2 * NB + 1))
    counts = [[0] * NB for _ in range(2)]
    for w in range(2):
        for b in range(NB):
            n = bounds[w * NB + b + 1] - bounds[w * NB + b]
            counts[w][b] = -(-n // NCORES)
    tot, wspan, subtiles, chunks = _layout(counts)

    nc = _build(tot, wspan, subtiles, chunks)

    ranges = {}
    off = 0
    for w in range(2):
        for b in range(NB):
            r = counts[w][b]
            if r:
                ranges[(w, b)] = off
            off += r
        off = -(-off // SUB) * SUB

    zT = np.zeros((128, NP), dtype=BF)
    zT[:, :N] = z.T.astype(BF)
    psiT = np.ascontiguousarray(psi.T).astype(BF)
    uc = np.zeros((NP, 64), dtype=BF)
    uc[:N] = c.astype(BF)
    u2i = np.zeros((NP, 128), dtype=BF)
    u2i[:N, 64:128] = c.astype(BF)
    u2a = np.ascontiguousarray(u2i[:WN0])
    u2b = np.ascontiguousarray(u2i[WN0:])

    in_maps = []
    slotmaps = []
    for i in range(NCORES):
        oh = np.zeros((128, tot), dtype=BF)
        didx = np.zeros(tot, dtype=np.int16)
        slot_edge = np.full(tot, -1, dtype=np.int64)
        for w in range(2):
            for b in range(NB):
                lo, hi = bounds[w * NB + b], bounds[w * NB + b + 1]
                if hi <= lo:
                    continue
                es = order[lo + i:hi:NCORES]
                if es.size == 0:
                    continue
                o = ranges[(w, b)]
                sl = np.arange(o, o + es.size)
                slot_edge[sl] = es
                oh[src[es] % 128, sl] = 1.0
                didx[sl] = (dst[es] & (WN0 - 1)).astype(np.int16)
        in_maps.append({
            "zT": zT, "psiT": psiT, "uc": uc, "u2a": u2a, "u2b": u2b,
            "oh": oh, "didx": _pack_idx16(didx), "w": weights,
        })
        slotmaps.append(slot_edge)
    return nc, in_maps, slotmaps


def unshard(results, slotmaps):
    final = np.empty(E, dtype=np.float32)
    for i in range(NCORES):
        vals = np.asarray(results[i]["out"])[0]
        se = slotmaps[i]
        mask = se >= 0
        final[se[mask]] = vals[mask]
    return final


def kernel(z, c, psi, weights, edge_index):
    nc, in_maps, slotmaps = prepare(z, c, psi, weights, edge_index)
    res = run_bass_kernel_spmd(nc, in_maps, core_ids=list(range(NCORES)))
    kernel.last_results = res
    return unshard(res.results, slotmaps)


kernel.last_results = None
